# revision 15
# baseline (speedup 1.0000x reference)
"""Trainium2 Bass kernel for nn_CapsuleLayer (dynamic routing capsule layer).

Reference computation (per batch element b):
    u_hat[b,r,c,o] = sum_i W[r,c,o,i] * x[b,r,i]        (R=1152, C=10, O=16, I=8)
    b_ij = 0
    3 routing iterations:
        c_ij = softmax(b_ij, axis=r)
        s_j[c,o] = sum_r c_ij[r,c] * u_hat[r,c,o]
        v = squash(s_j)  over o
        b_ij += sum_o u_hat[r,c,o] * v[c,o]   (except last iteration)
    output v -> [B, 1, C, O, 1]

Sharding: data-parallel over batch B=256 across 8 cores (32 each), W replicated.

Precision: build operands (Wp, xp, Df) in fp16 so u_hat carries ~1e-3 rel
error (bf16's 4e-3 on W and x dominated the old 2.3e-2 output error); the
s_j path uses a bf16 copy of W (Wq) + bf16 erm/yc since exp(b_ij) overflows
fp16 range.  u_hat storage stays fp16.

Layouts (host prepacks; partition index p = 8*rh + i; r = 16*ch + rh):
  - Df[p, ch, (b, rh')] : block-diag x, columns b-major so the psum drains
    into U1a/U1b4 are 16-element-contiguous on both sides.
  - build: pa = Wp[:,ch,0:128].T @ Df -> psum [co(c<8), (b, rh')];
    pb = Wp[:,ch,128:160].T @ Df -> [32, (b, rh')]; drains split DVE/ACT.
    s0 (iteration-0 s_j) accumulates in the same loop:
    xp[:,ch,:].T @ Wp[:,ch,:] -> psum [b, co], summed over all 72 chunks.
  - U1a[16c+o (c<8), b, r] fp16; U1b4[32bq+16cc+o, bg, r] fp16 (tail rows
    replicated per batch-quad so one 128-col FWL load serves 4 b's).
  - b_ij update r-major (per b, rc): lhsT = U1a[:, b, rc] (FWL), rhs = 8-col
    slice of block-diag(v) Vm1; tail batches 4 b's via U1b4/Vm24. One
    [128, 384] J-layout psum per rc accumulated into bsb by DVE adds.
  - softmax: per-rc ACT Exp (no max subtraction; logits stay < 40), Z via
    [128,1]-ones matmul -> pz [1, 320].
  - s_j: per ch: prep = EM64-expand matmul of 16 exp rows; ACT copies prep
    to bf16; yc = prep * xp (DVE, broadcast over c); two 128-col FWL
    matmuls Wq[:,ch,0:128] @ yc[:,0:160] and Wq[:,ch,32:160] @ yc[:,160:320]
    accumulate the c-diagonal into one [128, 320] psum (ps1|ps2 halves).
  - squash in [b, (c, o)] layout: 10 tiny PE transposes pull the s-diagonal
    into pt_s [32, 160]; Z transposed via 10 K=1 matmuls into [32, 10];
    Z^2-form squash (v = S*SS/((Z^2+SS)*sqrt(SS+eps*Z^2))) so no rzrep;
    reciprocal on [32, 10] costs ~90ns.  v transposed back (2 PE transposes
    + 10 copies) only when the next iteration needs Vm; the final v_b
    [32, 160] f32 DMAs straight out.
"""

import sys

if "/opt/trn_rl_repo" not in sys.path:
    sys.path.insert(0, "/opt/trn_rl_repo")

import numpy as np
import ml_dtypes

import concourse.bass as bass
import concourse.mybir as mybir
from concourse import bacc
from concourse.tile import TileContext

BF16 = mybir.dt.bfloat16
F16 = mybir.dt.float16
F32 = mybir.dt.float32
NPBF16 = ml_dtypes.bfloat16

B, R, C, O, I = 256, 1152, 10, 16, 8
NCORES = 8
BC = B // NCORES          # 32 batch elements per core
CH = R // 16              # 72 chunks of 16 r's
DGRP = 8                  # Df chunks per DMA
EPS = 1e-7
ExpF = mybir.ActivationFunctionType.Exp
SqrtF = mybir.ActivationFunctionType.Sqrt
ADD = mybir.AluOpType.add
AX_X = mybir.AxisListType.X


def _host_prep(xs, W):
    """Per-core input arrays. xs: [32,1152,8] f32, W: [1152,10,16,8] f32."""
    Wr = W.reshape(CH, 16, C, O, I).transpose(1, 4, 0, 2, 3)  # rh,i,ch,c,o
    Wp = Wr.reshape(128, CH * 160).astype(np.float16)
    Wq = Wr.reshape(128, CH * 160).astype(NPBF16)
    xr = xs.reshape(BC, CH, 16, I).transpose(2, 3, 1, 0)  # rh, i, ch, b
    xp = xr.reshape(128, CH * BC).astype(np.float16)
    xpb = xr.reshape(128, CH * BC).astype(NPBF16)
    tmp = xs.reshape(BC, CH, 16, I).transpose(1, 2, 3, 0)  # ch, rh, i, b
    D6 = np.zeros((CH, 16, I, BC, 16), np.float32)         # cols (b, rh')
    for rh in range(16):
        D6[:, rh, :, :, rh] = tmp[:, rh, :, :]
    Df = (D6.reshape(CH, 128, BC * 16).transpose(1, 0, 2)  # [128, CH, 512]
          .astype(np.float16))
    Df = np.ascontiguousarray(Df)
    return {"Wp": Wp, "Wq": Wq, "xp": xp, "xpb": xpb, "Df": Df}


def _host_consts():
    p = np.arange(128)
    # EM64[64k + 16*par + j2, par*128 + q] = (q//8 == j2): K=64 expand blocks
    # at partition bases 0/64; col group `par` selects which 16-row quarter of
    # the 64-block is replicated into the (rh, i) partition grid.
    EM64 = np.zeros((128, 4, 128), np.float32)
    for k in range(2):
        for par in range(4):
            for j2 in range(16):
                EM64[64 * k + 16 * par + j2, par, :] = (p // 8 == j2)
    EM64 = EM64.reshape(128, 512).astype(NPBF16)
    # EXP16[o', 16c+o] = (o'==o): replicate vD rows to all capsule rows
    EXP16 = np.tile(np.eye(16, dtype=np.float32), (1, 8)).astype(np.float16)
    # EXP24[o', 32bq+16cc+o] = (o'==o)
    EXP24 = np.tile(np.tile(np.eye(16, dtype=np.float32), (1, 2)),
                    (1, 4)).astype(np.float16)
    # MASK1[16c+o, c'*32+b] = (c==c') for c' < 8
    c_of = (p // 16)[:, None]
    cols = np.arange(256)[None, :] // 32
    MASK1 = (c_of == cols).astype(np.float16)
    # MASK24[32bq+16cc+o, bg*8+bq'*2+cc'] = (bq==bq') & (cc==cc')
    bq_of = (p // 32)[:, None]
    cc_of = ((p % 32) // 16)[:, None]
    col24 = np.arange(64)[None, :]
    MASK24 = ((bq_of == (col24 % 8) // 2) & (cc_of == col24 % 2)).astype(
        np.float16)
    ID32F = np.eye(32, dtype=np.float32)
    ID128F = np.eye(128, dtype=np.float32)
    return {"EM64": EM64, "EXP16": EXP16, "EXP24": EXP24, "MASK1": MASK1,
            "MASK24": MASK24, "ID32F": ID32F, "ID128F": ID128F}


def build_nc(stop_after=2):
    nc = bacc.Bacc("TRN2", target_bir_lowering=False, debug=False,
                   num_devices=NCORES)
    dr = {}
    for name, shape, dt in [
        ("Wp", [128, CH * 160], F16), ("Wq", [128, CH * 160], BF16),
        ("xp", [128, CH * BC], F16), ("xpb", [128, CH * BC], BF16),
        ("Df", [128, CH, 16 * BC], F16), ("EM64", [128, 512], BF16),
        ("EXP16", [16, 128], F16), ("EXP24", [16, 128], F16),
        ("MASK1", [128, 256], F16), ("MASK24", [128, 64], F16),
        ("ID32F", [32, 32], F32), ("ID128F", [128, 128], F32),
    ]:
        dr[name] = nc.dram_tensor(name, shape, dt, kind="ExternalInput").ap()
    d_out = nc.dram_tensor("out", [BC, 160], F32, kind="ExternalOutput").ap()

    with TileContext(nc) as tc:
        _emit(nc, tc, dr, d_out, stop_after)
    nc.compile()
    return nc


def _emit(nc, tc, dr, d_out, stop_after=2):
    from contextlib import ExitStack

    with ExitStack() as ctx:
        consts = ctx.enter_context(tc.tile_pool(name="consts", bufs=1))
        upool = ctx.enter_context(tc.tile_pool(name="upool", bufs=1))
        bpool = ctx.enter_context(tc.tile_pool(name="bpool", bufs=1))
        dstream = ctx.enter_context(tc.tile_pool(name="dstream", bufs=2))
        ystream = ctx.enter_context(tc.tile_pool(name="ystream", bufs=3))
        pstream = ctx.enter_context(tc.tile_pool(name="pstream", bufs=3))

        # ---- resident tiles ----
        Wp = consts.tile([128, CH, 160], F16)
        Wq = consts.tile([128, CH, 160], BF16)
        xp = consts.tile([128, CH, BC], F16)
        xpb = consts.tile([128, CH, BC], BF16)
        EM64 = consts.tile([128, 4, 128], BF16)
        EXP16 = consts.tile([16, 128], F16)
        EXP24 = consts.tile([16, 128], F16)
        MASK1 = consts.tile([128, 256], F16)
        MASK24 = consts.tile([128, 64], F16)
        ID32F = consts.tile([32, 32], F32)
        ID128F = consts.tile([128, 128], F32)
        onescol = consts.tile([128, 1], BF16)
        dumm = consts.tile([1, 1], F32)
        cEPS = consts.tile([32, 1], F32)    # squash epsilon

        for nm, t in [("Wp", Wp), ("Wq", Wq), ("xp", xp), ("xpb", xpb),
                      ("EM64", EM64)]:
            nc.sync.dma_start(out=t[:].rearrange("p a b -> p (a b)"),
                              in_=dr[nm])
        for nm, t in [("EXP16", EXP16), ("EXP24", EXP24), ("MASK1", MASK1),
                      ("MASK24", MASK24), ("ID32F", ID32F),
                      ("ID128F", ID128F)]:
            nc.sync.dma_start(out=t[:], in_=dr[nm])
        nc.gpsimd.memset(onescol[:], 1.0)
        nc.gpsimd.memset(dumm[:], 1.0)
        nc.gpsimd.memset(cEPS[:], EPS)

        U1a = upool.tile([128, BC, R], F16)     # [16c+o (c<8), b, r]
        U1b4 = upool.tile([128, 8, R], F16)     # [32bq+16cc+o, bg, r]

        bsb = bpool.tile([128, 9, 320], F32)    # b_ij r-major, cols J=c*32+b
        erm = bpool.tile([128, 9, 320], BF16)   # exp(b_ij)
        Vm1 = bpool.tile([128, 256], F16)       # block-diag v, cols c*32+b
        Vm24 = bpool.tile([128, 64], F16)       # [32bq+16cc+o, (bq',cc',bg)]

        # ============ Phase 1: build u_hat + iteration-0 s ============
        with tc.tile_pool(name="ppba", bufs=2, space="PSUM") as ppb1, \
             tc.tile_pool(name="ppbb", bufs=2, space="PSUM") as ppb2, \
             tc.tile_pool(name="pps0", bufs=1, space="PSUM") as pp_s0:
            ps0 = pp_s0.tile([32, 160], F32)
            for g in range(CH // DGRP):
                dfg = dstream.tile([128, DGRP, 16 * BC], F16, tag="dfg")
                nc.sync.dma_start(
                    out=dfg[:].rearrange("p a b -> p (a b)"),
                    in_=dr["Df"][:, g * DGRP:(g + 1) * DGRP, :].rearrange(
                        "p a b -> p (a b)"))
                for j in range(DGRP):
                    ch = g * DGRP + j
                    dfc = dfg[:, j, :]
                    pa = ppb1.tile([128, 16 * BC], F32, tag="pa")
                    pb = ppb2.tile([32, 16 * BC], F32, tag="pb")
                    nc.tensor.matmul(pa[:], Wp[:, ch, 0:128], dfc,
                                     start=True, stop=True)
                    nc.tensor.matmul(pb[:], Wp[:, ch, 128:160], dfc,
                                     start=True, stop=True)
                    nc.tensor.matmul(ps0[:], xp[:, ch, :], Wp[:, ch, :],
                                     start=(ch == 0), stop=(ch == CH - 1),
                                     skip_group_check=True)
                    # psum cols are (b, rh'); U1a cols are (b, r=16ch+rh')
                    pa_v = pa[:].rearrange("p (b h) -> p b h", h=16)
                    dst_a = U1a[:, :, 16 * ch:16 * ch + 16]
                    if ch % 2 == 0:
                        nc.vector.tensor_copy(dst_a, pa_v)
                    else:
                        nc.scalar.copy(dst_a, pa_v)
                    pb_v = pb[:].rearrange("p (b h) -> p b h", h=16)
                    for bq in range(4):
                        dst = U1b4[32 * bq:32 * bq + 32, :,
                                   16 * ch:16 * ch + 16]
                        src = pb_v[:, 8 * bq:8 * bq + 8, :]
                        if (bq % 2 == 0) == (ch % 2 == 0):
                            nc.scalar.copy(dst, src)
                        else:
                            nc.vector.tensor_copy(dst, src)

            # iteration 0: s0 is already [b, (c, o)]; Z-term = R (uniform c)
            stf0 = bpool.tile([32, 160], F32, tag="stf0")
            nc.scalar.copy(stf0[:], ps0[:])

        # routing-phase PSUM pools (opened after the build pools close)
        pp_bb = ctx.enter_context(tc.tile_pool(name="ppup", bufs=2,
                                               space="PSUM"))
        pp_z = ctx.enter_context(tc.tile_pool(name="ppz", bufs=1,
                                              space="PSUM"))
        pp_rep = ctx.enter_context(tc.tile_pool(name="pprep", bufs=2,
                                                space="PSUM"))
        pp_s = ctx.enter_context(tc.tile_pool(name="pps", bufs=1,
                                              space="PSUM"))
        pp_q = ctx.enter_context(tc.tile_pool(name="ppq", bufs=1,
                                              space="PSUM"))

        def squash_b(it, stf_raw, zb):
            """[32, (c, o)] squash.  stf_raw = raw S (f32 SBUF); zb [32, 10]
            f32 SBUF holds Z (None for it=0: Z=R).  Normalizes by Z FIRST --
            the (Z^2+SS)-form denominator overflows fp32 at iteration 2."""
            stf = bpool.tile([32, 160], F32, tag="stf")
            if it == 0:
                nc.scalar.mul(stf[:], stf_raw[:], 1.0 / R)
            else:
                rz = bpool.tile([32, 10, 1], F32, tag="rz")
                nc.vector.reciprocal(rz[:, :, 0], zb[:])
                nc.vector.tensor_mul(
                    stf[:].rearrange("p (c o) -> p c o", o=16),
                    stf_raw[:].rearrange("p (c o) -> p c o", o=16),
                    rz[:].broadcast_to([32, 10, 16]))
            sq2 = bpool.tile([32, 160], F32, tag="sq2")
            nc.vector.tensor_mul(sq2[:], stf[:], stf[:])
            SS = bpool.tile([32, 10], F32, tag="SS")
            nc.vector.tensor_reduce(
                SS[:], sq2[:].rearrange("p (c o) -> p c o", o=16),
                axis=AX_X, op=ADD)
            r1 = bpool.tile([32, 10], F32, tag="r1")
            nc.scalar.add(r1[:], SS[:], 1.0)
            r3 = bpool.tile([32, 10], F32, tag="r3")
            nc.scalar.add(r3[:], SS[:], cEPS[:])
            rt = bpool.tile([32, 10], F32, tag="rt")
            nc.scalar.activation(rt[:], r3[:], SqrtF)
            den = bpool.tile([32, 10], F32, tag="den")
            nc.vector.tensor_mul(den[:], r1[:], rt[:])
            inv = bpool.tile([32, 10], F32, tag="inv")
            nc.vector.reciprocal(inv[:], den[:])
            scl = bpool.tile([32, 10, 1], F32, tag="scl")
            nc.vector.tensor_mul(scl[:, :, 0], SS[:], inv[:])
            v_b = bpool.tile([32, 160], F32, tag="v_b")
            nc.vector.tensor_mul(
                v_b[:].rearrange("p (c o) -> p c o", o=16),
                stf[:].rearrange("p (c o) -> p c o", o=16),
                scl[:].broadcast_to([32, 10, 16]))
            return v_b

        def fill_vm(v_b):
            """v_b [32, 160] f16 -> vD [16, 320] -> Vm1/Vm24 (for b_up)."""
            ptv = pp_q.tile([128, 320], F32, tag="scr")
            for c in range(C):
                nc.tensor.transpose(ptv[0:16, 32 * c:32 * c + 32],
                                    v_b[:, 16 * c:16 * c + 16], ID32F[:])
            vD = bpool.tile([16, 320], F16, tag="vD")
            nc.vector.tensor_copy(vD[:], ptv[0:16, 0:320])
            ppv = pp_q.tile([128, 320], F32, tag="scr")
            nc.tensor.matmul(ppv[0:128, 0:256], EXP16[:], vD[:, 0:256],
                             start=True, stop=True)
            v24 = vD[:, 256:320].rearrange("p (c q g) -> p g q c", c=2, g=8)
            nc.tensor.matmul(ppv[0:128, 256:320], EXP24[:], v24,
                             start=True, stop=True)
            nc.vector.tensor_mul(Vm1[:], ppv[0:128, 0:256], MASK1[:])
            nc.vector.tensor_mul(Vm24[:], ppv[0:128, 256:320], MASK24[:])

        # -------- iteration 0 squash (stf0 copied out during build) --------
        v0 = squash_b(0, stf0, None)
        fill_vm(v0)

        # ================= iterations 1, 2 =================
        Vm1v = Vm1[:].rearrange("p (c b) -> p c b", b=BC)       # [128,8,32]
        for it in (1, 2):
            if it > stop_after:
                break
            psd = pp_s.tile([128, 320], F32, tag="psd")  # ps1|ps2
            pz = pp_z.tile([1, 320], F32, tag="pz")
            for rc in range(9):
                r0 = 128 * rc
                # ---- b_ij (+)= sum_o u_hat * v,  r-major PSUM ----
                pbb = pp_bb.tile([128, 384], F32, tag="pbb")
                pbv = pbb[:, 0:320].rearrange("p (c b) -> p c b", b=BC)
                for b in range(BC):
                    nc.tensor.matmul(pbv[:, 0:8, b],
                                     U1a[:, b, r0:r0 + 128], Vm1v[:, :, b],
                                     start=True, stop=True)
                for bg in range(8):
                    nc.tensor.matmul(
                        pbb[:, 320 + 8 * bg:328 + 8 * bg],
                        U1b4[:, bg, r0:r0 + 128],
                        Vm24[:, 8 * bg:8 * bg + 8],
                        start=True, stop=True)
                # tail cols (bg, bq', cc') -> J-cols 256 + cc'*32 + 8bq' + bg
                h2src = pbb[:, 320:384].rearrange(
                    "p (g q c) -> p g q c", q=4, c=2)
                h2dst = bsb[:, rc, 256:320].rearrange(
                    "p (c q g) -> p g q c", q=4, g=8)
                if it == 1:
                    nc.vector.tensor_copy(bsb[:, rc, 0:256], pbb[:, 0:256])
                    nc.vector.tensor_copy(h2dst, h2src)
                else:
                    nc.vector.tensor_add(bsb[:, rc, 0:256], bsb[:, rc, 0:256],
                                         pbb[:, 0:256])
                    nc.vector.tensor_add(h2dst, h2dst, h2src)
                # ---- softmax pieces for this rc ----
                nc.scalar.activation(erm[:, rc, :], bsb[:, rc, :], ExpF)
                nc.tensor.matmul(pz[:], onescol[:], erm[:, rc, :],
                                 start=(rc == 0), stop=(rc == 8),
                                 skip_group_check=True)
                # ---- s_j contributions from this rc's 8 chunks ----
                for q in range(8):
                    ch = 8 * rc + q
                    m, par = q // 4, q % 4
                    prep = pp_rep.tile([128, 320], F32, tag="prep")
                    nc.tensor.matmul(prep[:],
                                     EM64[64 * m:64 * m + 64, par, :],
                                     erm[64 * m:64 * m + 64, rc, :],
                                     start=True, stop=True)
                    prs = pstream.tile([128, 320], BF16, tag="prs")
                    nc.scalar.copy(prs[:], prep[:])
                    yc = ystream.tile([128, 320], BF16, tag="yc")
                    xb = xpb[:, ch:ch + 1, :].broadcast_to([128, 10, BC])
                    nc.vector.tensor_mul(
                        yc[:].rearrange("p (c b) -> p c b", b=BC),
                        prs[:].rearrange("p (c b) -> p c b", b=BC), xb)
                    # start exactly once per bank: a start clears has_written
                    # for the WHOLE bank, so a second start would orphan the
                    # first matmul's columns from the accumulation group.
                    nc.tensor.matmul(psd[:, 0:160],
                                     Wq[:, ch, 0:128], yc[:, 0:160],
                                     start=(ch == 0), stop=False,
                                     skip_group_check=True)
                    nc.tensor.matmul(psd[:, 160:320],
                                     Wq[:, ch, 32:160], yc[:, 160:320],
                                     start=False, stop=(ch == CH - 1),
                                     skip_group_check=True)
            # preload the Sqrt activation table off the critical path
            nc.scalar.activation(dumm[:], dumm[:], SqrtF)
            # ---- s-diagonal + Z into [32, *] layout via PE transposes ----
            # (engine APs may only start at partition 0/32/64/96, so slice
            # the IDENTITY's columns to pick diagonal rows instead of
            # slicing psd's partitions: in.T @ I[:, 16c:16c+16])
            sfull = bpool.tile([128, 320], F32, tag="sfull")
            nc.scalar.copy(sfull[:], psd[:])
            zz = bpool.tile([1, 320], F32, tag="zz")
            nc.vector.tensor_copy(zz[:], pz[:])
            pts = pp_q.tile([128, 320], F32, tag="scr")
            ptq = pts[0:32, 0:176]
            for c in range(C):
                if c < 5:
                    blk, row = sfull[:, 32 * c:32 * c + 32], 16 * c
                else:
                    blk = sfull[:, 160 + 32 * (c - 5):160 + 32 * (c - 5) + 32]
                    row = 16 * (c - 2)
                nc.tensor.transpose(ptq[:, 16 * c:16 * c + 16], blk,
                                    ID128F[:, row:row + 16])
            zz3 = zz[:].rearrange("p (c b) -> p c b", b=BC)
            for c in range(C):
                nc.tensor.matmul(ptq[:, 160 + c:161 + c], zz3[:, c, :],
                                 ID128F[0:1, 0:1], start=True, stop=True)
            stf_b = bpool.tile([32, 160], F32, tag="stf_b")
            nc.vector.tensor_copy(stf_b[:], ptq[:, 0:160])
            zb = bpool.tile([32, 10], F32, tag="zb")
            nc.vector.tensor_copy(zb[:], ptq[:, 160:170])
            v_b = squash_b(it, stf_b, zb)
            if it < 2:
                fill_vm(v_b)
            else:
                nc.sync.dma_start(out=d_out[:], in_=v_b[:])


_NC_CACHE = None


def _get_nc():
    global _NC_CACHE
    if _NC_CACHE is None:
        _NC_CACHE = build_nc()
    return _NC_CACHE


def decode_out(o):
    """[32, 160] core output (b, (c, o)) -> [32, 10, 16] (b, c, o)."""
    return o.reshape(BC, C, O)


def kernel(x, W):
    """Full-input entry point. x: [256,1152,8] f32, W: [1152,10,16,8] f32."""
    from concourse.bass_utils import run_bass_kernel_spmd

    x = np.asarray(x, np.float32)
    W = np.asarray(W, np.float32)
    nc = _get_nc()
    consts = _host_consts()
    in_maps = []
    for k in range(NCORES):
        m = _host_prep(x[k * BC:(k + 1) * BC], W)
        m.update(consts)
        in_maps.append(m)
    res = run_bass_kernel_spmd(nc, in_maps, core_ids=list(range(NCORES)))
    v = np.concatenate([decode_out(res.results[k]["out"])
                        for k in range(NCORES)], axis=0)  # [256, 10, 16]
    return v[:, None, :, :, None].astype(np.float32)
